# revision 13
# baseline (speedup 1.0000x reference)
"""MEGNet NodeModel on 8 Trainium2 NeuronCores (Bass/Tile).

Nodes are split into 8 contiguous blocks (12500/core); edges are bucketed
host-side by src node tile. Per 128-node tile, the first KI=4 edges of each
node go to "identity" slots (row = node-local index) so their scatter-add is
a matmul against a constant bf16 identity; only overflow edges use per-tile
indicator matrices built on VectorE (is_equal vs iota). The scatter-mean's
1/deg is pre-folded into the bf16 edge stream.

The 3-layer MLP runs feature-major in bf16 (f32 PSUM): per 512-col group,
z = W^T h via matmul, plus bias and u[batch] contributions injected by an
identity-weighted matmul into the same PSUM accumulation (keeps VectorE and
bias-free ACT relus off the critical path). BatchNorm batch stats are taken
by quarter-array sum (DVE accum) / square (ACT accum) passes overlapped with
the group loop, AllReduced as [128,2] per layer, and folded into the next
layer's weights/bias. The final BN affine writes a bf16 feature-major output
tensor; the host transposes per core during the unshard gather.
"""

import numpy as np
import ml_dtypes

from concourse import bacc, tile, mybir
from concourse import bass_utils

F32 = mybir.dt.float32
BF16 = mybir.dt.bfloat16
Alu = mybir.AluOpType
Act = mybir.ActivationFunctionType
BF16_NP = ml_dtypes.bfloat16

NCORES = 8
DIM = 128
TILE = 128
GRP = 4
N = 100000
E = 640000
B = 512
NPC = N // NCORES
NT = (NPC + TILE - 1) // TILE
W_LAST = NPC - (NT - 1) * TILE
BN_EPS = 1e-5
KI = 4                      # identity edge-slots per node


# ---------------------------------------------------------------- builder --

def build_program(nt, kr, w_last, n_total, ki=KI, reps=1, with_cc=True,
                  ncores=NCORES, stage=7, no_mbuild=False, plain_relu=False,
                  no_square=False, strm_bufs=6, ub_dve=False):
    """stage: 0 dma-only, 1 +segment, 3 +phase0, 5 +phase1, 7 full."""
    nc = bacc.Bacc("TRN2", target_bir_lowering=False, debug=False,
                   num_devices=ncores)
    kb = [ki + k for k in kr]
    toff = [0]
    for k in kb:
        toff.append(toff[-1] + k)
    ntile_tot = toff[-1]
    ngrp = (nt + GRP - 1) // GRP
    max_gk = max(toff[min((g + 1) * GRP, nt)] - toff[g * GRP]
                 for g in range(ngrp))

    edge_d = nc.dram_tensor("edge", [TILE, ntile_tot, DIM], BF16,
                            kind="ExternalInput")
    ir_d = nc.dram_tensor("ir", [TILE, ntile_tot], F32,
                          kind="ExternalInput")
    xT_d = nc.dram_tensor("xT", [DIM, nt * TILE], BF16, kind="ExternalInput")
    ubT_d = nc.dram_tensor("ubT", [DIM, nt * TILE], BF16,
                           kind="ExternalInput")
    iota_d = nc.dram_tensor("iota", [TILE, TILE], BF16, kind="ExternalInput")
    ident_d = nc.dram_tensor("ident", [TILE, TILE], F32, kind="ExternalInput")
    w0a_d = nc.dram_tensor("W0a", [DIM, DIM], BF16, kind="ExternalInput")
    w0b_d = nc.dram_tensor("W0b", [DIM, DIM], BF16, kind="ExternalInput")
    w1_d = nc.dram_tensor("W1", [DIM, DIM], F32, kind="ExternalInput")
    w2_d = nc.dram_tensor("W2", [DIM, DIM], F32, kind="ExternalInput")
    b1_d = nc.dram_tensor("b1", [DIM, 1], F32, kind="ExternalInput")
    b2_d = nc.dram_tensor("b2", [DIM, 1], F32, kind="ExternalInput")
    gb_d = nc.dram_tensor("gb", [DIM, 6], F32, kind="ExternalInput")
    out_d = nc.dram_tensor("out", [DIM, nt * TILE], BF16,
                           kind="ExternalOutput")

    def grp_tiles(g):
        return range(g * GRP, min((g + 1) * GRP, nt))

    def width(i):
        return w_last if i == nt - 1 else TILE

    def gwidth(g):
        return sum(width(i) for i in grp_tiles(g))

    with tile.TileContext(nc) as tc:
        with tc.tile_pool(name="const", bufs=1) as cst, \
             tc.tile_pool(name="rfull", bufs=1) as rpool, \
             tc.tile_pool(name="stat", bufs=1) as stat, \
             tc.tile_pool(name="stream", bufs=strm_bufs) as strm, \
             tc.tile_pool(name="work", bufs=3) as work, \
             tc.tile_pool(name="mpool", bufs=12) as mpool, \
             tc.tile_pool(name="ps_seg", bufs=4, space="PSUM") as ps_seg, \
             tc.tile_pool(name="ps_mm", bufs=3, space="PSUM") as ps_mm, \
             tc.tile_pool(name="ps_sm", bufs=1, space="PSUM") as ps_sm, \
             tc.tile_pool(name="dram", bufs=1, space="DRAM") as dram:

            iota_t = cst.tile([TILE, TILE], BF16, tag="iota")
            nc.sync.dma_start(out=iota_t[:], in_=iota_d[:])
            identb_t = cst.tile([TILE, TILE], BF16, tag="identb")
            nc.gpsimd.dma_start(out=identb_t[:], in_=ident_d[:])
            w0a_t = cst.tile([DIM, DIM], BF16, tag="w0a")
            nc.sync.dma_start(out=w0a_t[:], in_=w0a_d[:])
            w0b_t = cst.tile([DIM, DIM], BF16, tag="w0b")
            nc.sync.dma_start(out=w0b_t[:], in_=w0b_d[:])
            w1_t = cst.tile([DIM, DIM], F32, tag="w1")
            nc.sync.dma_start(out=w1_t[:], in_=w1_d[:])
            w2_t = cst.tile([DIM, DIM], F32, tag="w2")
            nc.sync.dma_start(out=w2_t[:], in_=w2_d[:])
            b1_t = cst.tile([DIM, 1], F32, tag="b1")
            nc.sync.dma_start(out=b1_t[:], in_=b1_d[:])
            b2_t = cst.tile([DIM, 1], F32, tag="b2")
            nc.sync.dma_start(out=b2_t[:], in_=b2_d[:])
            gb_t = cst.tile([DIM, 6], F32, tag="gb")
            nc.sync.dma_start(out=gb_t[:], in_=gb_d[:])
            ir_t = cst.tile([TILE, ntile_tot], F32, tag="ir")
            nc.sync.dma_start(out=ir_t[:], in_=ir_d[:])

            mconst_t = cst.tile([TILE, TILE], BF16, tag="mconst")
            nc.vector.tensor_scalar(out=mconst_t[:], in0=iota_t[:],
                                    scalar1=ir_t[:, 0:1], scalar2=None,
                                    op0=Alu.is_equal)
            ones_t = cst.tile([DIM, GRP * TILE], BF16, tag="ones")
            nc.vector.memset(ones_t[:], 1.0)
            eps_t = cst.tile([DIM, 1], F32, tag="eps")
            nc.vector.memset(eps_t[:], BN_EPS)
            cc_in = dram.tile([DIM, 2], F32, tag="cc_in")
            cc_out = dram.tile([DIM, 2], F32, tag="cc_out")
            npc = (nt - 1) * TILE + w_last
            # stat-pass split points: after these groups, accumulate the
            # columns since the previous split (hides stat passes under the
            # remaining groups' compute)
            SPLITS = [3, 7, 11, 15, 19, ngrp - 1]
            SCOLS = []
            prev = 0
            for sg in SPLITS:
                end = min((sg + 1) * GRP * TILE, npc)
                SCOLS.append(slice(prev, end))
                prev = end

            def cross_core_stats(loc, tag):
                nc.sync.dma_start(out=cc_in[:], in_=loc[:])
                if with_cc:
                    nc.gpsimd.collective_compute(
                        "AllReduce", Alu.add,
                        replica_groups=[list(range(ncores))],
                        ins=[cc_in[:].opt()], outs=[cc_out[:].opt()])
                    src = cc_out
                else:
                    src = cc_in
                gs = stat.tile([DIM, 2], F32, tag=f"gs{tag}")
                nc.sync.dma_start(out=gs[:], in_=src[:])
                return gs

            def bn_affine(gs, layer):
                g_ap = gb_t[:, 2 * layer:2 * layer + 1]
                be_ap = gb_t[:, 2 * layer + 1:2 * layer + 2]
                t = stat.tile([DIM, 4], F32, tag=f"bn{layer}")
                me, var, istd = t[:, 0:2], t[:, 2:3], t[:, 3:4]
                mean, ex2 = t[:, 0:1], t[:, 1:2]
                nc.vector.tensor_scalar(out=me, in0=gs[:, 0:2],
                                        scalar1=1.0 / n_total, scalar2=None,
                                        op0=Alu.mult)
                nc.vector.tensor_tensor(out=var, in0=mean, in1=mean,
                                        op=Alu.mult)
                nc.vector.tensor_tensor(out=var, in0=ex2, in1=var,
                                        op=Alu.subtract)
                nc.scalar.activation(out=var, in_=var, func=Act.Sqrt,
                                     bias=eps_t[:])
                nc.vector.reciprocal(out=istd, in_=var)
                ac = stat.tile([DIM, 2], F32, tag=f"ac{layer}")
                a_ap, c_ap = ac[:, 0:1], ac[:, 1:2]
                nc.vector.tensor_tensor(out=a_ap, in0=g_ap, in1=istd,
                                        op=Alu.mult)
                nc.vector.tensor_tensor(out=c_ap, in0=a_ap, in1=mean,
                                        op=Alu.mult)
                nc.vector.tensor_tensor(out=c_ap, in0=be_ap, in1=c_ap,
                                        op=Alu.subtract)
                return a_ap, c_ap

            def fold_bn(a_ap, c_ap, w_t, b_t, layer):
                ws = stat.tile([DIM, DIM], BF16, tag=f"ws{layer}")
                nc.vector.tensor_scalar(out=ws[:], in0=w_t[:], scalar1=a_ap,
                                        scalar2=None, op0=Alu.mult)
                psb = ps_sm.tile([DIM, 1], F32, tag="psb")
                nc.tensor.matmul(psb[:], lhsT=w_t[:], rhs=c_ap,
                                 start=True, stop=True)
                bp = stat.tile([DIM, 1], F32, tag=f"bp{layer}")
                nc.vector.tensor_tensor(out=bp[:], in0=psb[:], in1=b_t[:],
                                        op=Alu.add)
                # broadcast bias along the free axis so it can be added into
                # PSUM by an identity matmul (keeps relu bias-free)
                bB = stat.tile([DIM, GRP * TILE], BF16, tag=f"bB{layer}")
                nc.vector.tensor_scalar(out=bB[:], in0=ones_t[:],
                                        scalar1=bp[:], scalar2=None,
                                        op0=Alu.mult)
                return ws, bB

            def stats_part(r_out, dum, scrb, locq, cols, tag):
                # partial stat passes over a column range: DVE sum-accum +
                # ACT square-accum run in parallel on separate engines
                lq = stat.tile([DIM, 2], F32, tag=f"lq{tag}")
                if plain_relu:
                    nc.vector.memset(lq[:, 0:1], 1.0)
                else:
                    nc.vector.tensor_scalar(out=dum[:, cols],
                                            in0=r_out[:, cols],
                                            scalar1=1.0, scalar2=0.0,
                                            op0=Alu.mult, op1=Alu.add,
                                            accum_out=lq[:, 0:1])
                if no_square:
                    nc.vector.memset(lq[:, 1:2], 1.0)
                else:
                    nc.scalar.activation(out=scrb[:, cols],
                                         in_=r_out[:, cols],
                                         func=Act.Square,
                                         accum_out=lq[:, 1:2])
                locq.append(lq)

            def stats_combine(locq, loc, tag):
                # pairwise tree-reduce the partial [DIM,2] stats into loc
                lvl, i = list(locq), 0
                while len(lvl) > 2:
                    nxt = []
                    for j in range(0, len(lvl) - 1, 2):
                        t = stat.tile([DIM, 2], F32, tag=f"cm{tag}_{i}")
                        i += 1
                        nc.vector.tensor_tensor(out=t[:], in0=lvl[j][:],
                                                in1=lvl[j + 1][:], op=Alu.add)
                        nxt.append(t)
                    if len(lvl) % 2:
                        nxt.append(lvl[-1])
                    lvl = nxt
                nc.vector.tensor_tensor(out=loc[:], in0=lvl[0][:],
                                        in1=lvl[1][:], op=Alu.add)

            def mlp_phase(r_in, r_out, ws, bB, dum, scrb, loc, ptag):
                locq = []
                for g in range(ngrp):
                    wg = gwidth(g)
                    sl = slice(g * GRP * TILE, g * GRP * TILE + wg)
                    ps = ps_mm.tile([DIM, GRP * TILE], F32, tag="ps")
                    nc.tensor.matmul(ps[:, :wg], lhsT=ws[:], rhs=r_in[:, sl],
                                     start=True, stop=False)
                    nc.tensor.matmul(ps[:, :wg], lhsT=identb_t[:],
                                     rhs=bB[:, :wg], start=False, stop=True)
                    nc.scalar.activation(out=r_out[:, sl], in_=ps[:, :wg],
                                         func=Act.Relu)
                    if g in SPLITS:
                        qi = SPLITS.index(g)
                        stats_part(r_out, dum, scrb, locq, SCOLS[qi],
                                   f"{ptag}{qi}")
                stats_combine(locq, loc, ptag)

            def body(rep):
                if stage >= 2:
                    r0 = rpool.tile([DIM, nt * TILE], BF16, tag="r0")
                if stage >= 3:
                    r1 = rpool.tile([DIM, nt * TILE], BF16, tag="r1")
                    scr_big = rpool.tile([DIM, nt * TILE], BF16, tag="scrb")
                    loc0 = stat.tile([DIM, 2], F32, tag="loc0")
                    locq0 = []

                # ---------------- phase 0: segment mean + layer 0 ----------
                for g in range(ngrp):
                    wg = gwidth(g)
                    tiles = list(grp_tiles(g))
                    sl = slice(g * GRP * TILE, g * GRP * TILE + wg)
                    gk0, gk1 = toff[tiles[0]], toff[tiles[-1] + 1]
                    attr = strm.tile([TILE, max_gk * DIM], BF16, tag="attr")
                    nc.sync.dma_start(
                        out=attr[:, :(gk1 - gk0) * DIM],
                        in_=edge_d[:, gk0:gk1, :])
                    xt = strm.tile([DIM, GRP * TILE], BF16, tag="xt")
                    nc.sync.dma_start(out=xt[:, :wg], in_=xT_d[:, sl])
                    ubt = strm.tile([DIM, GRP * TILE], BF16, tag="ubt")
                    nc.sync.dma_start(out=ubt[:, :wg], in_=ubT_d[:, sl])
                    if stage < 1:
                        continue

                    ve = work.tile([DIM, GRP * TILE], BF16, tag="ve")
                    for j, i in enumerate(tiles):
                        psA = ps_seg.tile([DIM, TILE], F32, tag="psA")
                        nkb = kb[i]
                        for k in range(nkb):
                            t_idx = toff[i] + k
                            if k < ki or no_mbuild:
                                m = identb_t if k < ki else mconst_t
                            else:
                                m = mpool.tile([TILE, TILE], BF16, tag="m")
                                nc.vector.tensor_scalar(
                                    out=m[:], in0=iota_t[:],
                                    scalar1=ir_t[:, t_idx:t_idx + 1],
                                    scalar2=None, op0=Alu.is_equal)
                            nc.tensor.matmul(
                                psA[:],
                                lhsT=attr[:, (t_idx - gk0) * DIM:
                                          (t_idx - gk0 + 1) * DIM],
                                rhs=m[:], start=(k == 0),
                                stop=(k == nkb - 1))
                        nc.scalar.activation(
                            out=ve[:, j * TILE:j * TILE + width(i)],
                            in_=psA[:, :width(i)], func=Act.Copy)
                    if stage < 2:
                        continue

                    ps0 = ps_mm.tile([DIM, GRP * TILE], F32, tag="ps")
                    if ub_dve:
                        nc.tensor.matmul(ps0[:, :wg], lhsT=w0a_t[:],
                                         rhs=xt[:, :wg], start=True,
                                         stop=False)
                        nc.tensor.matmul(ps0[:, :wg], lhsT=w0b_t[:],
                                         rhs=ve[:, :wg], start=False,
                                         stop=True)
                        nc.vector.tensor_tensor(out=ps0[:, :wg],
                                                in0=ps0[:, :wg],
                                                in1=ubt[:, :wg], op=Alu.add)
                    else:
                        nc.tensor.matmul(ps0[:, :wg], lhsT=w0a_t[:],
                                         rhs=xt[:, :wg], start=True,
                                         stop=False)
                        nc.tensor.matmul(ps0[:, :wg], lhsT=w0b_t[:],
                                         rhs=ve[:, :wg], start=False,
                                         stop=False)
                        nc.tensor.matmul(ps0[:, :wg], lhsT=identb_t[:],
                                         rhs=ubt[:, :wg], start=False,
                                         stop=True)
                    nc.scalar.activation(out=r0[:, sl], in_=ps0[:, :wg],
                                         func=Act.Relu)
                    if stage >= 3 and g in SPLITS:
                        qi = SPLITS.index(g)
                        stats_part(r0, r1, scr_big, locq0, SCOLS[qi],
                                   f"p0{qi}")
                if stage < 3:
                    return
                stats_combine(locq0, loc0, "p0")
                if stage < 5:
                    return

                gs0 = cross_core_stats(loc0, "0")
                a0, c0 = bn_affine(gs0, 0)
                w1s, b1B = fold_bn(a0, c0, w1_t, b1_t, 1)
                if stage == 4:   # chain cost probe: one consumer group only
                    ps = ps_mm.tile([DIM, GRP * TILE], F32, tag="ps")
                    nc.tensor.matmul(ps[:], lhsT=w1s[:], rhs=r0[:, :GRP * TILE],
                                     start=True, stop=True)
                    nc.scalar.activation(out=r1[:, :GRP * TILE], in_=ps[:],
                                         func=Act.Relu)
                    return

                # ---------------- phase 1 ----------------------------------
                loc1 = stat.tile([DIM, 2], F32, tag="loc1")
                mlp_phase(r0, r1, w1s, b1B, r0, scr_big, loc1, "p1")
                if stage < 7:
                    return

                gs1 = cross_core_stats(loc1, "1")
                a1, c1 = bn_affine(gs1, 1)
                w2s, b2B = fold_bn(a1, c1, w2_t, b2_t, 2)

                # ---------------- phase 2 (r2 overwrites r0) ---------------
                r2 = r0
                loc2 = stat.tile([DIM, 2], F32, tag="loc2")
                mlp_phase(r1, r2, w2s, b2B, r1, scr_big, loc2, "p2")

                gs2 = cross_core_stats(loc2, "2")
                a2, c2 = bn_affine(gs2, 2)

                # ---------- epilogue: BN2 affine, feature-major store ------
                # (final [node, dim] layout restored on the host during the
                #  gather: out.T per core)
                outw = rpool.tile([DIM, nt * TILE], BF16, tag="outw")
                for ci in range(0, ngrp, 7):
                    for g in range(ci, min(ngrp, ci + 7)):
                        wg = gwidth(g)
                        sl = slice(g * GRP * TILE, g * GRP * TILE + wg)
                        nc.vector.tensor_scalar(
                            out=outw[:, sl], in0=r2[:, sl], scalar1=a2,
                            scalar2=c2, op0=Alu.mult, op1=Alu.add)
                    lo = ci * GRP * TILE
                    hi = min(min(ngrp, ci + 7) * GRP * TILE, npc)
                    nc.sync.dma_start(out=out_d[:, lo:hi],
                                      in_=outw[:, lo:hi])

            if reps == 1:
                body(0)
            else:
                with tc.For_i(0, reps):
                    body(0)

    nc.compile()
    return nc


# ------------------------------------------------------------ host side ---

def preprocess(x, edge_index, edge_attr, u, batch,
               W0, b0, W1, b1, W2, b2, g0, be0, g1, be1, g2, be2,
               ncores=NCORES, npc=NPC, ki=KI):
    """Shard + lay out inputs. Returns (in_maps, kr) with kr the per-node-tile
    remainder (indicator) tile counts, shared across cores."""
    x = np.asarray(x, dtype=np.float32)
    edge_attr = np.asarray(edge_attr, dtype=np.float32)
    u = np.asarray(u, dtype=np.float32)
    W0 = np.asarray(W0, dtype=np.float32)
    src = np.asarray(edge_index)[0].astype(np.int64)
    batch_i = np.asarray(batch).astype(np.int64)
    n, dim = x.shape
    e = src.shape[0]
    nt = (npc + TILE - 1) // TILE

    deg = np.bincount(src, minlength=n).astype(np.int64)
    recip = (1.0 / np.maximum(deg, 1.0)).astype(np.float32)

    perm = np.argsort(src, kind="stable")
    src_s = src[perm]
    attr_scaled = edge_attr[perm] * recip[src_s][:, None]

    node_starts = np.concatenate([[0], np.cumsum(deg)[:-1]])
    jrank = np.arange(e) - node_starts[src_s]

    core_of = src_s // npc
    local = src_s % npc
    ltile = local // TILE
    lc = (local % TILE).astype(np.int64)

    is_id = jrank < ki
    # remainder sequencing per (core, node-tile)
    rem = ~is_id
    rem_bucket = (core_of * nt + ltile)[rem]
    rem_counts = np.bincount(rem_bucket, minlength=ncores * nt)
    kr = np.ceil(rem_counts.reshape(ncores, nt).max(axis=0)
                 / TILE).astype(np.int64)
    rem_starts = np.concatenate([[0], np.cumsum(rem_counts)[:-1]])
    rem_seq = np.arange(rem.sum()) - rem_starts[rem_bucket]

    kb = ki + kr
    toff = np.concatenate([[0], np.cumsum(kb)])[:-1]     # [nt]
    ntile_tot = int(ki * nt + kr.sum())

    # flat slot per edge (within its core's layout)
    slot = np.empty(e, np.int64)
    slot[is_id] = (toff[ltile[is_id]] + jrank[is_id]) * TILE + lc[is_id]
    slot[rem] = ((toff[ltile[rem]] + ki + rem_seq // TILE) * TILE
                 + rem_seq % TILE)

    ubias = (u @ W0[2 * DIM:3 * DIM, :] + np.asarray(b0, np.float32))[batch_i]

    iota = np.broadcast_to(np.arange(TILE, dtype=BF16_NP),
                           (TILE, TILE)).copy()
    ident = np.eye(TILE, dtype=np.float32)
    gb = np.stack([np.asarray(v, np.float32) for v in
                   (g0, be0, g1, be1, g2, be2)], axis=1)
    common = {
        "iota": iota, "ident": ident,
        "W0a": W0[0:DIM, :].astype(BF16_NP),
        "W0b": W0[DIM:2 * DIM, :].astype(BF16_NP),
        "W1": np.asarray(W1, np.float32), "W2": np.asarray(W2, np.float32),
        "b1": np.asarray(b1, np.float32).reshape(DIM, 1),
        "b2": np.asarray(b2, np.float32).reshape(DIM, 1),
        "gb": gb,
    }

    in_maps = []
    for c in range(ncores):
        msk = core_of == c
        attr_pad = np.zeros((ntile_tot * TILE, dim), BF16_NP)
        attr_pad[slot[msk]] = attr_scaled[msk].astype(BF16_NP)
        attr_l = np.ascontiguousarray(
            attr_pad.reshape(ntile_tot, TILE, dim).transpose(1, 0, 2))
        ir = np.full((ntile_tot * TILE,), -1.0, np.float32)
        mr = msk & rem
        ir[slot[mr]] = lc[mr].astype(np.float32)
        ir_l = np.ascontiguousarray(ir.reshape(ntile_tot, TILE).T)

        lo, hi = c * npc, (c + 1) * npc
        xt = np.zeros((DIM, nt * TILE), BF16_NP)
        xt[:, :npc] = x[lo:hi].astype(BF16_NP).T
        ubt = np.zeros((DIM, nt * TILE), BF16_NP)
        ubt[:, :npc] = ubias[lo:hi].astype(BF16_NP).T
        in_maps.append({"edge": attr_l, "ir": ir_l, "xT": xt, "ubT": ubt,
                        **common})
    return in_maps, tuple(int(k) for k in kr)


_CACHE = {}


def _get_program(kr, n_total, nt, w_last):
    key = (kr, n_total, nt, w_last)
    if key not in _CACHE:
        _CACHE[key] = build_program(nt, kr, w_last, n_total,
                                    reps=1, with_cc=True)
    return _CACHE[key]


def kernel(**inputs):
    in_maps, kr = preprocess(**inputs)
    nc = _get_program(kr, N, NT, W_LAST)
    res = bass_utils.run_bass_kernel_spmd(
        nc, in_maps, core_ids=list(range(NCORES)))
    out = np.concatenate(
        [res.results[c]["out"][:, :NPC].T.astype(np.float32)
         for c in range(NCORES)], axis=0)
    return out


# revision 16
# speedup vs baseline: 1.0330x; 1.0330x over previous
"""MEGNet NodeModel on 8 Trainium2 NeuronCores (Bass/Tile).

Nodes are split into 8 contiguous blocks (12500/core); edges are bucketed
host-side by src node tile. Per 128-node tile, the first KI=4 edges of each
node go to "identity" slots (row = node-local index) so their scatter-add is
a matmul against a constant bf16 identity; only overflow edges use per-tile
indicator matrices built on VectorE (is_equal vs iota). The scatter-mean's
1/deg is pre-folded into the bf16 edge stream.

The 3-layer MLP runs feature-major in bf16 (f32 PSUM): per 512-col group,
z = W^T h via matmul, plus bias and u[batch] contributions injected by an
identity-weighted matmul into the same PSUM accumulation (keeps VectorE and
bias-free ACT relus off the critical path). BatchNorm batch stats are taken
by quarter-array sum (DVE accum) / square (ACT accum) passes overlapped with
the group loop, AllReduced as [128,2] per layer, and folded into the next
layer's weights/bias. The final BN affine writes a bf16 feature-major output
tensor; the host transposes per core during the unshard gather.
"""

import numpy as np
import ml_dtypes

from concourse import bacc, tile, mybir
from concourse import bass_utils

F32 = mybir.dt.float32
BF16 = mybir.dt.bfloat16
Alu = mybir.AluOpType
Act = mybir.ActivationFunctionType
BF16_NP = ml_dtypes.bfloat16

NCORES = 8
DIM = 128
TILE = 128
GRP = 4
N = 100000
E = 640000
B = 512
NPC = N // NCORES
NT = (NPC + TILE - 1) // TILE
W_LAST = NPC - (NT - 1) * TILE
BN_EPS = 1e-5
KI = 4                      # identity edge-slots per node


# ---------------------------------------------------------------- builder --

def build_program(nt, kr, w_last, n_total, ki=KI, reps=1, with_cc=True,
                  ncores=NCORES, stage=7, no_mbuild=False, plain_relu=False,
                  no_square=False, strm_bufs=6, ub_dve=False):
    """stage: 0 dma-only, 1 +segment, 3 +phase0, 5 +phase1, 7 full."""
    nc = bacc.Bacc("TRN2", target_bir_lowering=False, debug=False,
                   num_devices=ncores)
    kb = [ki + k for k in kr]
    toff = [0]
    for k in kb:
        toff.append(toff[-1] + k)
    ntile_tot = toff[-1]
    ngrp = (nt + GRP - 1) // GRP
    max_gk = max(toff[min((g + 1) * GRP, nt)] - toff[g * GRP]
                 for g in range(ngrp))

    edge_d = nc.dram_tensor("edge", [TILE, ntile_tot, DIM], BF16,
                            kind="ExternalInput")
    ir_d = nc.dram_tensor("ir", [TILE, ntile_tot], F32,
                          kind="ExternalInput")
    xT_d = nc.dram_tensor("xT", [DIM, nt * TILE], BF16, kind="ExternalInput")
    ubT_d = nc.dram_tensor("ubT", [DIM, nt * TILE], BF16,
                           kind="ExternalInput")
    iota_d = nc.dram_tensor("iota", [TILE, TILE], BF16, kind="ExternalInput")
    ident_d = nc.dram_tensor("ident", [TILE, TILE], F32, kind="ExternalInput")
    w0a_d = nc.dram_tensor("W0a", [DIM, DIM], BF16, kind="ExternalInput")
    w0b_d = nc.dram_tensor("W0b", [DIM, DIM], BF16, kind="ExternalInput")
    w1_d = nc.dram_tensor("W1", [DIM, DIM], F32, kind="ExternalInput")
    w2_d = nc.dram_tensor("W2", [DIM, DIM], F32, kind="ExternalInput")
    b1_d = nc.dram_tensor("b1", [DIM, 1], F32, kind="ExternalInput")
    b2_d = nc.dram_tensor("b2", [DIM, 1], F32, kind="ExternalInput")
    gb_d = nc.dram_tensor("gb", [DIM, 6], F32, kind="ExternalInput")
    out_d = nc.dram_tensor("out", [DIM, nt * TILE], BF16,
                           kind="ExternalOutput")

    def grp_tiles(g):
        return range(g * GRP, min((g + 1) * GRP, nt))

    def width(i):
        return w_last if i == nt - 1 else TILE

    def gwidth(g):
        return sum(width(i) for i in grp_tiles(g))

    with tile.TileContext(nc) as tc:
        with tc.tile_pool(name="const", bufs=1) as cst, \
             tc.tile_pool(name="rfull", bufs=1) as rpool, \
             tc.tile_pool(name="stat", bufs=1) as stat, \
             tc.tile_pool(name="stream", bufs=strm_bufs) as strm, \
             tc.tile_pool(name="work", bufs=3) as work, \
             tc.tile_pool(name="mpool", bufs=12) as mpool, \
             tc.tile_pool(name="ps_seg", bufs=4, space="PSUM") as ps_seg, \
             tc.tile_pool(name="ps_mm", bufs=3, space="PSUM") as ps_mm, \
             tc.tile_pool(name="ps_sm", bufs=1, space="PSUM") as ps_sm, \
             tc.tile_pool(name="dram", bufs=1, space="DRAM") as dram:

            iota_t = cst.tile([TILE, TILE], BF16, tag="iota")
            nc.sync.dma_start(out=iota_t[:], in_=iota_d[:])
            identb_t = cst.tile([TILE, TILE], BF16, tag="identb")
            nc.gpsimd.dma_start(out=identb_t[:], in_=ident_d[:])
            w0a_t = cst.tile([DIM, DIM], BF16, tag="w0a")
            nc.sync.dma_start(out=w0a_t[:], in_=w0a_d[:])
            w0b_t = cst.tile([DIM, DIM], BF16, tag="w0b")
            nc.sync.dma_start(out=w0b_t[:], in_=w0b_d[:])
            w1_t = cst.tile([DIM, DIM], F32, tag="w1")
            nc.sync.dma_start(out=w1_t[:], in_=w1_d[:])
            w2_t = cst.tile([DIM, DIM], F32, tag="w2")
            nc.sync.dma_start(out=w2_t[:], in_=w2_d[:])
            b1_t = cst.tile([DIM, 1], F32, tag="b1")
            nc.sync.dma_start(out=b1_t[:], in_=b1_d[:])
            b2_t = cst.tile([DIM, 1], F32, tag="b2")
            nc.sync.dma_start(out=b2_t[:], in_=b2_d[:])
            gb_t = cst.tile([DIM, 6], F32, tag="gb")
            nc.sync.dma_start(out=gb_t[:], in_=gb_d[:])
            ir_t = cst.tile([TILE, ntile_tot], F32, tag="ir")
            nc.sync.dma_start(out=ir_t[:], in_=ir_d[:])

            mconst_t = cst.tile([TILE, TILE], BF16, tag="mconst")
            nc.vector.tensor_scalar(out=mconst_t[:], in0=iota_t[:],
                                    scalar1=ir_t[:, 0:1], scalar2=None,
                                    op0=Alu.is_equal)
            ones_t = cst.tile([DIM, GRP * TILE], BF16, tag="ones")
            nc.vector.memset(ones_t[:], 1.0)
            eps_t = cst.tile([DIM, 1], F32, tag="eps")
            nc.vector.memset(eps_t[:], BN_EPS)
            cc_in = dram.tile([DIM, 2], F32, tag="cc_in")
            cc_out = dram.tile([DIM, 2], F32, tag="cc_out")
            npc = (nt - 1) * TILE + w_last
            # stat-pass split points: after these groups, accumulate the
            # columns since the previous split (hides stat passes under the
            # remaining groups' compute)
            SPLITS = [6, 12, 18, ngrp - 1]
            SCOLS = []
            prev = 0
            for sg in SPLITS:
                end = min((sg + 1) * GRP * TILE, npc)
                SCOLS.append(slice(prev, end))
                prev = end

            def cross_core_stats(loc, tag):
                nc.sync.dma_start(out=cc_in[:], in_=loc[:])
                if with_cc:
                    nc.gpsimd.collective_compute(
                        "AllReduce", Alu.add,
                        replica_groups=[list(range(ncores))],
                        ins=[cc_in[:].opt()], outs=[cc_out[:].opt()])
                    src = cc_out
                else:
                    src = cc_in
                gs = stat.tile([DIM, 2], F32, tag=f"gs{tag}")
                nc.sync.dma_start(out=gs[:], in_=src[:])
                return gs

            def bn_affine(gs, layer):
                g_ap = gb_t[:, 2 * layer:2 * layer + 1]
                be_ap = gb_t[:, 2 * layer + 1:2 * layer + 2]
                t = stat.tile([DIM, 4], F32, tag=f"bn{layer}")
                me, var, istd = t[:, 0:2], t[:, 2:3], t[:, 3:4]
                mean, ex2 = t[:, 0:1], t[:, 1:2]
                nc.vector.tensor_scalar(out=me, in0=gs[:, 0:2],
                                        scalar1=1.0 / n_total, scalar2=None,
                                        op0=Alu.mult)
                nc.vector.tensor_tensor(out=var, in0=mean, in1=mean,
                                        op=Alu.mult)
                nc.vector.tensor_tensor(out=var, in0=ex2, in1=var,
                                        op=Alu.subtract)
                nc.scalar.activation(out=var, in_=var, func=Act.Sqrt,
                                     bias=eps_t[:])
                nc.vector.reciprocal(out=istd, in_=var)
                ac = stat.tile([DIM, 2], F32, tag=f"ac{layer}")
                a_ap, c_ap = ac[:, 0:1], ac[:, 1:2]
                nc.vector.tensor_tensor(out=a_ap, in0=g_ap, in1=istd,
                                        op=Alu.mult)
                nc.vector.tensor_tensor(out=c_ap, in0=a_ap, in1=mean,
                                        op=Alu.mult)
                nc.vector.tensor_tensor(out=c_ap, in0=be_ap, in1=c_ap,
                                        op=Alu.subtract)
                return a_ap, c_ap

            def fold_bn(a_ap, c_ap, w_t, b_t, layer):
                ws = stat.tile([DIM, DIM], BF16, tag=f"ws{layer}")
                nc.vector.tensor_scalar(out=ws[:], in0=w_t[:], scalar1=a_ap,
                                        scalar2=None, op0=Alu.mult)
                psb = ps_sm.tile([DIM, 1], F32, tag="psb")
                nc.tensor.matmul(psb[:], lhsT=w_t[:], rhs=c_ap,
                                 start=True, stop=True)
                bp = stat.tile([DIM, 1], F32, tag=f"bp{layer}")
                nc.vector.tensor_tensor(out=bp[:], in0=psb[:], in1=b_t[:],
                                        op=Alu.add)
                # broadcast bias along the free axis so it can be added into
                # PSUM by an identity matmul (keeps relu bias-free)
                bB = stat.tile([DIM, GRP * TILE], BF16, tag=f"bB{layer}")
                nc.vector.tensor_scalar(out=bB[:], in0=ones_t[:],
                                        scalar1=bp[:], scalar2=None,
                                        op0=Alu.mult)
                return ws, bB

            def stats_part(r_out, dum, scrb, locq, cols, tag):
                # partial stat passes over a column range: DVE sum-accum +
                # ACT square-accum run in parallel on separate engines
                lq = stat.tile([DIM, 2], F32, tag=f"lq{tag}")
                if plain_relu:
                    nc.vector.memset(lq[:, 0:1], 1.0)
                else:
                    nc.vector.tensor_scalar(out=dum[:, cols],
                                            in0=r_out[:, cols],
                                            scalar1=1.0, scalar2=0.0,
                                            op0=Alu.mult, op1=Alu.add,
                                            accum_out=lq[:, 0:1])
                if no_square:
                    nc.vector.memset(lq[:, 1:2], 1.0)
                else:
                    nc.scalar.activation(out=scrb[:, cols],
                                         in_=r_out[:, cols],
                                         func=Act.Square,
                                         accum_out=lq[:, 1:2])
                locq.append(lq)

            def stats_combine(locq, loc, tag):
                ab = stat.tile([DIM, 2], F32, tag=f"ab{tag}")
                cd = stat.tile([DIM, 2], F32, tag=f"cd{tag}")
                nc.vector.tensor_tensor(out=ab[:], in0=locq[0][:],
                                        in1=locq[1][:], op=Alu.add)
                nc.vector.tensor_tensor(out=cd[:], in0=locq[2][:],
                                        in1=locq[3][:], op=Alu.add)
                nc.vector.tensor_tensor(out=loc[:], in0=ab[:], in1=cd[:],
                                        op=Alu.add)

            def mlp_phase(r_in, r_out, ws, bB, dum, scrb, loc, ptag):
                locq = []
                for g in range(ngrp):
                    wg = gwidth(g)
                    sl = slice(g * GRP * TILE, g * GRP * TILE + wg)
                    ps = ps_mm.tile([DIM, GRP * TILE], F32, tag="ps")
                    nc.tensor.matmul(ps[:, :wg], lhsT=ws[:], rhs=r_in[:, sl],
                                     start=True, stop=False)
                    nc.tensor.matmul(ps[:, :wg], lhsT=identb_t[:],
                                     rhs=bB[:, :wg], start=False, stop=True)
                    nc.scalar.activation(out=r_out[:, sl], in_=ps[:, :wg],
                                         func=Act.Relu)
                    if g in SPLITS:
                        qi = SPLITS.index(g)
                        stats_part(r_out, dum, scrb, locq, SCOLS[qi],
                                   f"{ptag}{qi}")
                stats_combine(locq, loc, ptag)

            def body(rep):
                if stage >= 2:
                    r0 = rpool.tile([DIM, nt * TILE], BF16, tag="r0")
                if stage >= 3:
                    r1 = rpool.tile([DIM, nt * TILE], BF16, tag="r1")
                    scr_big = rpool.tile([DIM, nt * TILE], BF16, tag="scrb")
                    loc0 = stat.tile([DIM, 2], F32, tag="loc0")
                    locq0 = []

                # ---------------- phase 0: segment mean + layer 0 ----------
                for g in range(ngrp):
                    wg = gwidth(g)
                    tiles = list(grp_tiles(g))
                    sl = slice(g * GRP * TILE, g * GRP * TILE + wg)
                    gk0, gk1 = toff[tiles[0]], toff[tiles[-1] + 1]
                    attr = strm.tile([TILE, max_gk * DIM], BF16, tag="attr")
                    nc.sync.dma_start(
                        out=attr[:, :(gk1 - gk0) * DIM],
                        in_=edge_d[:, gk0:gk1, :])
                    xt = strm.tile([DIM, GRP * TILE], BF16, tag="xt")
                    nc.sync.dma_start(out=xt[:, :wg], in_=xT_d[:, sl])
                    ubt = strm.tile([DIM, GRP * TILE], BF16, tag="ubt")
                    nc.sync.dma_start(out=ubt[:, :wg], in_=ubT_d[:, sl])
                    if stage < 1:
                        continue

                    ve = work.tile([DIM, GRP * TILE], BF16, tag="ve")
                    for j, i in enumerate(tiles):
                        psA = ps_seg.tile([DIM, TILE], F32, tag="psA")
                        nkb = kb[i]
                        for k in range(nkb):
                            t_idx = toff[i] + k
                            if k < ki or no_mbuild:
                                m = identb_t if k < ki else mconst_t
                            else:
                                m = mpool.tile([TILE, TILE], BF16, tag="m")
                                nc.vector.tensor_scalar(
                                    out=m[:], in0=iota_t[:],
                                    scalar1=ir_t[:, t_idx:t_idx + 1],
                                    scalar2=None, op0=Alu.is_equal)
                            nc.tensor.matmul(
                                psA[:],
                                lhsT=attr[:, (t_idx - gk0) * DIM:
                                          (t_idx - gk0 + 1) * DIM],
                                rhs=m[:], start=(k == 0),
                                stop=(k == nkb - 1))
                        nc.scalar.activation(
                            out=ve[:, j * TILE:j * TILE + width(i)],
                            in_=psA[:, :width(i)], func=Act.Copy)
                    if stage < 2:
                        continue

                    ps0 = ps_mm.tile([DIM, GRP * TILE], F32, tag="ps")
                    if ub_dve:
                        nc.tensor.matmul(ps0[:, :wg], lhsT=w0a_t[:],
                                         rhs=xt[:, :wg], start=True,
                                         stop=False)
                        nc.tensor.matmul(ps0[:, :wg], lhsT=w0b_t[:],
                                         rhs=ve[:, :wg], start=False,
                                         stop=True)
                        nc.vector.tensor_tensor(out=ps0[:, :wg],
                                                in0=ps0[:, :wg],
                                                in1=ubt[:, :wg], op=Alu.add)
                    else:
                        nc.tensor.matmul(ps0[:, :wg], lhsT=w0a_t[:],
                                         rhs=xt[:, :wg], start=True,
                                         stop=False)
                        nc.tensor.matmul(ps0[:, :wg], lhsT=w0b_t[:],
                                         rhs=ve[:, :wg], start=False,
                                         stop=False)
                        nc.tensor.matmul(ps0[:, :wg], lhsT=identb_t[:],
                                         rhs=ubt[:, :wg], start=False,
                                         stop=True)
                    nc.scalar.activation(out=r0[:, sl], in_=ps0[:, :wg],
                                         func=Act.Relu)
                    if stage >= 3 and g in SPLITS:
                        qi = SPLITS.index(g)
                        stats_part(r0, r1, scr_big, locq0, SCOLS[qi],
                                   f"p0{qi}")
                if stage < 3:
                    return
                stats_combine(locq0, loc0, "p0")
                if stage < 5:
                    return

                gs0 = cross_core_stats(loc0, "0")
                a0, c0 = bn_affine(gs0, 0)
                w1s, b1B = fold_bn(a0, c0, w1_t, b1_t, 1)
                if stage == 4:   # chain cost probe: one consumer group only
                    ps = ps_mm.tile([DIM, GRP * TILE], F32, tag="ps")
                    nc.tensor.matmul(ps[:], lhsT=w1s[:], rhs=r0[:, :GRP * TILE],
                                     start=True, stop=True)
                    nc.scalar.activation(out=r1[:, :GRP * TILE], in_=ps[:],
                                         func=Act.Relu)
                    return

                # ---------------- phase 1 ----------------------------------
                loc1 = stat.tile([DIM, 2], F32, tag="loc1")
                mlp_phase(r0, r1, w1s, b1B, r0, scr_big, loc1, "p1")
                if stage < 7:
                    return

                gs1 = cross_core_stats(loc1, "1")
                a1, c1 = bn_affine(gs1, 1)
                w2s, b2B = fold_bn(a1, c1, w2_t, b2_t, 2)

                # ---------------- phase 2 (r2 overwrites r0) ---------------
                r2 = r0
                loc2 = stat.tile([DIM, 2], F32, tag="loc2")
                mlp_phase(r1, r2, w2s, b2B, r1, scr_big, loc2, "p2")

                gs2 = cross_core_stats(loc2, "2")
                a2, c2 = bn_affine(gs2, 2)

                # ---------- epilogue: BN2 affine, feature-major store ------
                # (final [node, dim] layout restored on the host during the
                #  gather: out.T per core)
                outw = rpool.tile([DIM, nt * TILE], BF16, tag="outw")
                for ci in range(0, ngrp, 7):
                    for g in range(ci, min(ngrp, ci + 7)):
                        wg = gwidth(g)
                        sl = slice(g * GRP * TILE, g * GRP * TILE + wg)
                        nc.vector.tensor_scalar(
                            out=outw[:, sl], in0=r2[:, sl], scalar1=a2,
                            scalar2=c2, op0=Alu.mult, op1=Alu.add)
                    lo = ci * GRP * TILE
                    hi = min(min(ngrp, ci + 7) * GRP * TILE, npc)
                    nc.sync.dma_start(out=out_d[:, lo:hi],
                                      in_=outw[:, lo:hi])

            if reps == 1:
                body(0)
            else:
                with tc.For_i(0, reps):
                    body(0)

    nc.compile()
    return nc


# ------------------------------------------------------------ host side ---

def preprocess(x, edge_index, edge_attr, u, batch,
               W0, b0, W1, b1, W2, b2, g0, be0, g1, be1, g2, be2,
               ncores=NCORES, npc=NPC, ki=KI):
    """Shard + lay out inputs. Returns (in_maps, kr) with kr the per-node-tile
    remainder (indicator) tile counts, shared across cores."""
    x = np.asarray(x, dtype=np.float32)
    edge_attr = np.asarray(edge_attr, dtype=np.float32)
    u = np.asarray(u, dtype=np.float32)
    W0 = np.asarray(W0, dtype=np.float32)
    src = np.asarray(edge_index)[0].astype(np.int64)
    batch_i = np.asarray(batch).astype(np.int64)
    n, dim = x.shape
    e = src.shape[0]
    nt = (npc + TILE - 1) // TILE

    deg = np.bincount(src, minlength=n).astype(np.int64)
    recip = (1.0 / np.maximum(deg, 1.0)).astype(np.float32)

    perm = np.argsort(src, kind="stable")
    src_s = src[perm]
    attr_scaled = edge_attr[perm] * recip[src_s][:, None]

    node_starts = np.concatenate([[0], np.cumsum(deg)[:-1]])
    jrank = np.arange(e) - node_starts[src_s]

    core_of = src_s // npc
    local = src_s % npc
    ltile = local // TILE
    lc = (local % TILE).astype(np.int64)

    is_id = jrank < ki
    # remainder sequencing per (core, node-tile)
    rem = ~is_id
    rem_bucket = (core_of * nt + ltile)[rem]
    rem_counts = np.bincount(rem_bucket, minlength=ncores * nt)
    kr = np.ceil(rem_counts.reshape(ncores, nt).max(axis=0)
                 / TILE).astype(np.int64)
    rem_starts = np.concatenate([[0], np.cumsum(rem_counts)[:-1]])
    rem_seq = np.arange(rem.sum()) - rem_starts[rem_bucket]

    kb = ki + kr
    toff = np.concatenate([[0], np.cumsum(kb)])[:-1]     # [nt]
    ntile_tot = int(ki * nt + kr.sum())

    # flat slot per edge (within its core's layout)
    slot = np.empty(e, np.int64)
    slot[is_id] = (toff[ltile[is_id]] + jrank[is_id]) * TILE + lc[is_id]
    slot[rem] = ((toff[ltile[rem]] + ki + rem_seq // TILE) * TILE
                 + rem_seq % TILE)

    ubias = (u @ W0[2 * DIM:3 * DIM, :] + np.asarray(b0, np.float32))[batch_i]

    iota = np.broadcast_to(np.arange(TILE, dtype=BF16_NP),
                           (TILE, TILE)).copy()
    ident = np.eye(TILE, dtype=np.float32)
    gb = np.stack([np.asarray(v, np.float32) for v in
                   (g0, be0, g1, be1, g2, be2)], axis=1)
    common = {
        "iota": iota, "ident": ident,
        "W0a": W0[0:DIM, :].astype(BF16_NP),
        "W0b": W0[DIM:2 * DIM, :].astype(BF16_NP),
        "W1": np.asarray(W1, np.float32), "W2": np.asarray(W2, np.float32),
        "b1": np.asarray(b1, np.float32).reshape(DIM, 1),
        "b2": np.asarray(b2, np.float32).reshape(DIM, 1),
        "gb": gb,
    }

    in_maps = []
    for c in range(ncores):
        msk = core_of == c
        attr_pad = np.zeros((ntile_tot * TILE, dim), BF16_NP)
        attr_pad[slot[msk]] = attr_scaled[msk].astype(BF16_NP)
        attr_l = np.ascontiguousarray(
            attr_pad.reshape(ntile_tot, TILE, dim).transpose(1, 0, 2))
        ir = np.full((ntile_tot * TILE,), -1.0, np.float32)
        mr = msk & rem
        ir[slot[mr]] = lc[mr].astype(np.float32)
        ir_l = np.ascontiguousarray(ir.reshape(ntile_tot, TILE).T)

        lo, hi = c * npc, (c + 1) * npc
        xt = np.zeros((DIM, nt * TILE), BF16_NP)
        xt[:, :npc] = x[lo:hi].astype(BF16_NP).T
        ubt = np.zeros((DIM, nt * TILE), BF16_NP)
        ubt[:, :npc] = ubias[lo:hi].astype(BF16_NP).T
        in_maps.append({"edge": attr_l, "ir": ir_l, "xT": xt, "ubT": ubt,
                        **common})
    return in_maps, tuple(int(k) for k in kr)


_CACHE = {}


def _get_program(kr, n_total, nt, w_last):
    key = (kr, n_total, nt, w_last)
    if key not in _CACHE:
        _CACHE[key] = build_program(nt, kr, w_last, n_total,
                                    reps=1, with_cc=True)
    return _CACHE[key]


def kernel(**inputs):
    in_maps, kr = preprocess(**inputs)
    nc = _get_program(kr, N, NT, W_LAST)
    res = bass_utils.run_bass_kernel_spmd(
        nc, in_maps, core_ids=list(range(NCORES)))
    out = np.concatenate(
        [res.results[c]["out"][:, :NPC].T.astype(np.float32)
         for c in range(NCORES)], axis=0)
    return out
